# revision 48
# baseline (speedup 1.0000x reference)
"""Trainium2 Bass kernel for single-head attention with pre-softmax score dropout.

Reference computation (per batch element b):
    qp = q @ Wq.T; kp = k @ Wk.T; vp = v @ Wv.T      (biases are zero)
    S  = (qp @ kp.T) / sqrt(D) * drop_mask
    out = softmax(S, axis=-1) @ vp

Sharding: data-parallel over batch B=8 across the 8 NeuronCores (one batch
element per core); weights replicated. No collectives.

Host-side prep (layout only — no activation FLOPs): inputs are shipped
pre-transposed and pre-cast to bf16 (qT/kT/vT), and the two score
projections are constant-folded into one matrix Nw = Wq^T @ Wk (weight-weight
algebra, f32 on host), plus WvT = Wv^T.  This removes every TensorE transpose
and halves HBM traffic vs f32.

Device pipeline per core — TensorE runs ONLY productive N~512 matmuls:
  - qmT[b,t] = sum_a Nw[a,b] qT[a,t]          (64 matmuls)
  - S^T tiles [tk=128, tq=512]: lhsT=kT slice, rhs=qmT chunk (256 matmuls).
    Scores are computed TRANSPOSED so that exp() directly yields P^T, which is
    exactly the stationary operand the PV matmul needs — no P transpose.
  - DVE multiplies S^T by maskT tile; ScalarE computes exp(x/sqrt(D)) -> bf16.
  - vp[t,e] with a ones-column appended ([128, 513] bf16).  PV for each tq
    block runs two accumulation chains: chain A (cols E1:D + ones) first,
    then chain B (cols 0:E1).  The ones-column row-sum drops out of chain A,
    so the reciprocal + chain-A normalize overlap chain B's matmuls and only
    one normalize + a 128KB bf16 store trail the final matmul.

Softmax max-subtraction is skipped deliberately: scores are ~N(0,1) scaled by
at most 1/(1-p)=1.43, so |s| stays far inside f32 exp range.

DMA plan.  Key hardware mechanics (measured): in-flight ring transfers on a
queue share its bandwidth concurrently (~125-150GB/s per HWDGE queue, ~358GB/s
core budget); the completion-semaphore ring (8 sems) is shared across both
HWDGE queues in emission order; a DMA-issue instruction that waits for a ring
slot or a dependency blocks the whole in-order engine queue behind it; DMA
rate needs 2-4KB/partition contiguous descriptors (1KB is ~4x slower).  Hence:
  sync   : qT0, kT pieces g0/g1/g4..g7, wvT, vT half 1, qT2, qT3, [out blocks]
  scalar : Nw, qT1, kT g2/g3, vT half 0 and nothing later — Scalar also runs
           the qmT copies / exp / normalize, so its queue must never carry a
           late-blocking DMA issue.
  gpsimd : mask quarter-chunks [P,4,TCH], fp8 {0,1} in DRAM cast to bf16 in
           flight (mask scale is folded into Nw on the host); mask pool of 4
           bufs makes chunk c+2's loads chain on chunk c's consumption.
kT is re-blocked in DRAM as tkb-PAIR pieces (each its own contiguous
2KB/partition region) split across both queues, so score chunk 0 is never
DMA-stalled.  The phase order qm0,sc0,qm1,sc1,vp,pv0,qm2,sc2,pv1,... pushes
the vT/wvT deadline to ~46us (pt pool holds two chunks of P^T tiles).
A 46-matmul warm-up burns the initial Nw/qT0 DMA wait and holds the PE
clock-ramp (HAM governor) at speed before productive work starts.
"""

import numpy as np
import ml_dtypes

import concourse.bass as bass
import concourse.bacc as bacc
import concourse.mybir as mybir
import concourse.tile as tile
from concourse.bass_utils import run_bass_kernel_spmd

B, T, D, P = 8, 2048, 512, 128
DB = D // P     # 4 blocks of the contraction/projection dims
TB = T // P     # 16 tk row blocks
NCH = 4         # tq chunks
TCH = T // NCH  # 512
QT = 4          # mask quarter: 4 tk-blocks per quarter tile
E1 = 272        # PV split: chain B = [0:E1]; chain A = [E1:D]+ones
F32 = mybir.dt.float32
BF16 = mybir.dt.bfloat16
AF = mybir.ActivationFunctionType
INV_SQRT_D = 1.0 / float(np.sqrt(D))
BF = ml_dtypes.bfloat16

_CACHED = {}


def _build():
    nc = bacc.Bacc("TRN2", target_bir_lowering=False, debug=False, num_devices=B)

    FP8 = mybir.dt.float8e4
    qT_ext = nc.declare_dram_parameter("qTb", [NCH, P, DB, TCH], BF16,
                                       isOutput=False)
    # kT in tkb-pair pieces: [g, p, bb, j2] = k[g*(TCH//2)+j2, bb*P+p]
    kT_ext = nc.declare_dram_parameter("kTb", [2 * NCH, P, DB, TCH // 2], BF16,
                                       isOutput=False)
    vT_ext = nc.declare_dram_parameter("vT", [D, T], BF16, isOutput=False)
    n_ext = nc.declare_dram_parameter("Nwb", [P, DB, DB, P], BF16,
                                      isOutput=False)
    wvT_ext = nc.declare_dram_parameter("WvTb", [P, DB, D], BF16,
                                        isOutput=False)
    # mask, quarter-blocked: [c, p, quarter, j, tq] = maskT[(q*4+j)*P+p, c*TCH+tq]
    mT_ext = nc.declare_dram_parameter("maskT", [NCH, P, NCH, QT, TCH], FP8,
                                       isOutput=False)
    out_ext = nc.declare_dram_parameter("out", [T, D], BF16, isOutput=True)

    with tile.TileContext(nc) as tc:
        with (
            tc.tile_pool(name="wsb", bufs=1) as wsb_pool,
            tc.tile_pool(name="xsb", bufs=1) as xsb_pool,
            tc.tile_pool(name="mask", bufs=4) as mask_pool,
            tc.tile_pool(name="pm", bufs=4) as pm_pool,
            tc.tile_pool(name="pt", bufs=34) as pt_pool,
            tc.tile_pool(name="ob", bufs=3) as ob_pool,
            tc.tile_pool(name="small", bufs=4) as small_pool,
            tc.tile_pool(name="psw", bufs=2, space="PSUM") as psw_pool,
            # pv pools need only 1 buf each: opA's consumers (recip+mulA) run
            # during the opB chain and vice versa, so the next tqb's chain
            # never waits.  The freed banks give the score phase 4 sp bufs,
            # decoupling the PE from DVE/mask-DMA latency.
            tc.tile_pool(name="pss", bufs=4, space="PSUM") as pss_pool,
            tc.tile_pool(name="pso1", bufs=1, space="PSUM") as pso1_pool,
            tc.tile_pool(name="pso2", bufs=1, space="PSUM") as pso2_pool,
        ):
            # ---- DMA in (see module docstring for the pacing plan).
            n_sb = wsb_pool.tile([P, DB, DB, P], BF16, tag="n")
            qT_sb = xsb_pool.tile([P, NCH, DB, TCH], BF16, tag="qT")
            # kT pair-major so each pair DMA is 2KB/partition contiguous on
            # BOTH sides (a strided dst would degrade descriptors to 512B).
            kT_sb = xsb_pool.tile([P, 2 * NCH, DB, TCH // 2], BF16, tag="kT")
            TH2 = TCH // 2

            def load_ktp(g, eng):
                eng.dma_start(kT_sb[:, g], kT_ext[g])

            # In-flight ring transfers on a queue share its bandwidth and
            # the Tile scheduler freely hoists DMA issues, so exact pacing
            # is not controllable — instead keep qT0/Nw first on their
            # queues, spread the kT pieces across both queues in consumption
            # order, and put vT0 on scalar so the vp phase is never gated by
            # the long sync-queue backlog.
            nc.sync.dma_start(qT_sb[:, 0], qT_ext[0])
            nc.scalar.dma_start(n_sb[:], n_ext[:])
            load_ktp(0, nc.sync)
            nc.scalar.dma_start(qT_sb[:, 1], qT_ext[1])
            load_ktp(1, nc.sync)
            load_ktp(2, nc.scalar)
            load_ktp(3, nc.scalar)
            load_ktp(4, nc.sync)
            load_ktp(5, nc.sync)
            load_ktp(6, nc.sync)
            load_ktp(7, nc.sync)
            vT_sb = xsb_pool.tile([P, DB, T], BF16, tag="vT")
            TH = T // 2
            nc.scalar.dma_start(
                vT_sb[:, :, 0:TH],
                vT_ext[0:D, 0:TH].rearrange("(a p) t -> p a t", p=P),
            )
            wvT_sb = wsb_pool.tile([P, DB, D], BF16, tag="wvT")
            nc.sync.dma_start(wvT_sb[:], wvT_ext[:])
            nc.sync.dma_start(
                vT_sb[:, :, TH:T],
                vT_ext[0:D, TH:T].rearrange("(a p) t -> p a t", p=P),
            )
            nc.sync.dma_start(qT_sb[:, 2], qT_ext[2])
            nc.sync.dma_start(qT_sb[:, 3], qT_ext[3])

            masks = {}

            def load_masks(c):
                for q in range(NCH):
                    mk = mask_pool.tile([P, QT, TCH], BF16, tag="mask")
                    nc.gpsimd.dma_start(mk[:], mT_ext[c, :, q])
                    masks[(c, q)] = mk

            load_masks(0)
            load_masks(1)

            qmT_sb = xsb_pool.tile([P, DB, T], BF16, tag="qmT")
            vp_sb = xsb_pool.tile([P, TB, D + 1], BF16, tag="vp")
            nc.vector.memset(vp_sb[:, :, D:D + 1], 1.0)

            # ---- PE warm-up: the HAM clock gate holds the PE at reduced
            # clock until it has seen ~3us of sustained activity, and TensorE
            # is idle waiting on the Nw/qT0 DMA until ~11us anyway — burn the
            # wait on dummy matmuls (memset data, no DRAM dep).
            warm = wsb_pool.tile([P, P], BF16, tag="warm")
            nc.vector.memset(warm[:], 0.5)
            wps = psw_pool.tile([P, P], F32, tag="work")
            for i in range(46):
                nc.tensor.matmul(
                    wps[:], warm[:], warm[:], start=(i == 0), stop=(i == 45)
                )

            def qm_chunk(c):
                for bb in range(DB):
                    work = psw_pool.tile([P, TCH], F32, tag="work")
                    for ab in range(DB):
                        nc.tensor.matmul(
                            work[:],
                            n_sb[:, bb, ab, :],
                            qT_sb[:, c, ab, :],
                            start=(ab == 0),
                            stop=(ab == DB - 1),
                        )
                    # alternate engines so consecutive copies overlap; Vector
                    # first — after a score chunk Vector (muls) drains ~1.4us
                    # before Scalar (exps), so the first copy lands sooner
                    dst = qmT_sb[:, bb, c * TCH:(c + 1) * TCH]
                    if bb % 2 == 0:
                        nc.vector.tensor_copy(dst, work[:])
                    else:
                        nc.scalar.copy(dst, work[:])

            def score_chunk(c):
                """Returns the 16 P^T tiles [tk=128, tq=TCH] for this chunk."""
                if c + 2 < NCH:
                    load_masks(c + 2)
                pts = []
                for tkb in range(TB):
                    sp = pss_pool.tile([P, TCH], F32, tag="sp")
                    for bb in range(DB):
                        nc.tensor.matmul(
                            sp[:],
                            kT_sb[:, tkb // 2, bb,
                                  (tkb % 2) * P:(tkb % 2 + 1) * P],
                            qmT_sb[:, bb, c * TCH:(c + 1) * TCH],
                            start=(bb == 0),
                            stop=(bb == DB - 1),
                        )
                    pmt = pm_pool.tile([P, TCH], F32, tag="pm")
                    nc.vector.tensor_mul(
                        pmt[:], sp[:], masks[(c, tkb // QT)][:, tkb % QT, :]
                    )
                    pt = pt_pool.tile([P, TCH], BF16, tag="pt")
                    nc.scalar.activation(pt[:], pmt[:], AF.Exp, scale=INV_SQRT_D)
                    pts.append(pt)
                return pts

            def vp_phase():
                for tb in range(TB):
                    work = psw_pool.tile([P, D], F32, tag="work")
                    for db in range(DB):
                        nc.tensor.matmul(
                            work[:],
                            vT_sb[:, db, tb * P:(tb + 1) * P],
                            wvT_sb[:, db, :],
                            start=(db == 0),
                            stop=(db == DB - 1),
                        )
                    nc.vector.tensor_copy(vp_sb[:, tb, 0:D], work[:])

            def pv_chunk(c, pts):
                for tqb in range(NCH):
                    m = c * NCH + tqb
                    # chain A first: cols [E1:D] + the ones column, so the
                    # softmax row-sum (last col) is ready while chain B runs.
                    opA = pso2_pool.tile([P, D - E1 + 1], F32, tag="opA")
                    for tkb in range(TB):
                        nc.tensor.matmul(
                            opA[:], pts[tkb][:, tqb * P:(tqb + 1) * P],
                            vp_sb[:, tkb, E1:D + 1],
                            start=(tkb == 0), stop=(tkb == TB - 1),
                        )
                    rinv = small_pool.tile([P, 1], F32, tag="rinv")
                    nc.vector.reciprocal(rinv[:], opA[:, D - E1:D - E1 + 1])
                    ob = ob_pool.tile([P, D], BF16, tag="ob")
                    nc.scalar.mul(ob[:, E1:D], opA[:, 0:D - E1], rinv[:, 0:1])
                    opB = pso1_pool.tile([P, E1], F32, tag="opB")
                    for tkb in range(TB):
                        nc.tensor.matmul(
                            opB[:], pts[tkb][:, tqb * P:(tqb + 1) * P],
                            vp_sb[:, tkb, 0:E1],
                            start=(tkb == 0), stop=(tkb == TB - 1),
                        )
                    # NOTE: splitting this final normalize across Scalar +
                    # Vector (tensor_scalar_mul with an AP scalar) and the
                    # store across both queues by partition halves looked
                    # like a ~1us tail win but reproducibly slowed EVERY
                    # matmul in the kernel by ~20% (sc 216->259ns) — some
                    # SBUF-port / clock-governor interaction.  Keep the
                    # simple single-engine normalize + one contiguous store.
                    nc.scalar.mul(ob[:, 0:E1], opB[:], rinv[:, 0:1])
                    nc.sync.dma_start(out_ext[m * P:(m + 1) * P, :], ob[:])

            # ---- schedule: keep TensorE dense; sc1 before vp relaxes the
            # vT/wvT DMA deadline; pv(c) recycles sc(c)'s pt tiles just in
            # time for sc(c+2)'s allocations (pt pool holds 2 chunks + slack).
            qm_chunk(0)
            pts0 = score_chunk(0)
            qm_chunk(1)
            pts1 = score_chunk(1)
            vp_phase()
            pv_chunk(0, pts0)
            qm_chunk(2)
            pts2 = score_chunk(2)
            pv_chunk(1, pts1)
            qm_chunk(3)
            pts3 = score_chunk(3)
            pv_chunk(2, pts2)
            pv_chunk(3, pts3)

    nc.compile()
    return nc


def get_nc(fast=True):
    key = "fast"
    if key not in _CACHED:
        _CACHED[key] = _build()
    return _CACHED[key]


def make_in_maps_fast(q, k, v, Wq, Wk, Wv, drop_mask):
    q = np.asarray(q, np.float32)
    k = np.asarray(k, np.float32)
    v = np.asarray(v, np.float32)

    def _block(w):
        # [D, D] -> [P, DB, D] with [p, a, :] = w[a*P + p, :]
        return np.ascontiguousarray(
            w.reshape(DB, P, D).transpose(1, 0, 2).astype(BF)
        )

    def _chunk_block(xT):
        # [D, T] -> [c, p, ab, j] = xT[ab*P + p, c*TCH + j]
        return np.ascontiguousarray(
            xT.reshape(DB, P, NCH, TCH).transpose(2, 1, 0, 3)
        )

    def _pair_block(xT):
        # [D, T] -> [g, p, bb, j2] = xT[bb*P + p, g*(TCH//2) + j2]
        return np.ascontiguousarray(
            xT.reshape(DB, P, 2 * NCH, TCH // 2).transpose(2, 1, 0, 3)
        )

    dm = np.asarray(drop_mask, np.float32)
    # drop_mask is {0, 1/(1-p)}: fold its scale into Nw (weight algebra)
    # and ship the mask itself as exact {0,1} fp8.
    mask_scale = float(dm.max()) if dm.size else 1.0
    if mask_scale == 0.0:
        mask_scale = 1.0
    # [a, b] -> [p, bb, ab, j] = Nw[ab*P + p, bb*P + j]  (4KB/partition DMA)
    Nw_f = (
        np.asarray(Wq, np.float32).T @ np.asarray(Wk, np.float32)
    ) * np.float32(mask_scale)
    Nw = np.ascontiguousarray(
        Nw_f.reshape(DB, P, DB, P).transpose(1, 2, 0, 3).astype(BF)
    )
    WvT = _block(np.asarray(Wv, np.float32).T.copy())
    F8 = ml_dtypes.float8_e4m3
    maps = []
    for i in range(B):
        # maskT[tk, tq] -> [c, p, quarter, j, tq], tk = (quarter*QT+j)*P + p
        mt = (dm[i].T != 0).astype(F8)
        mtb = np.ascontiguousarray(
            mt.reshape(NCH, QT, P, NCH, TCH).transpose(3, 2, 0, 1, 4)
        )
        maps.append({
            "qTb": _chunk_block(q[i].T.astype(BF)),
            "kTb": _pair_block(k[i].T.astype(BF)),
            "vT": np.ascontiguousarray(v[i].T.astype(BF)),
            "Nwb": Nw,
            "WvTb": WvT,
            "maskT": mtb,
        })
    return maps


def _numpy_reference(q, k, v, Wq, bq, Wk, bk, Wv, bv, drop_mask):
    """Correctness fallback for nonzero biases (never hit by setup_inputs)."""
    qp = np.einsum("btd,ed->bte", q, Wq) + bq
    kp = np.einsum("btd,ed->bte", k, Wk) + bk
    vp = np.einsum("btd,ed->bte", v, Wv) + bv
    score = np.einsum("bqd,bkd->bqk", qp, kp) / np.sqrt(np.float32(D))
    score = score * drop_mask
    score -= score.max(axis=-1, keepdims=True)
    e = np.exp(score)
    attn = e / e.sum(axis=-1, keepdims=True)
    return np.einsum("bqk,bkd->bqd", attn, vp).astype(np.float32)


def kernel(q, k, v, Wq, bq, Wk, bk, Wv, bv, drop_mask):
    zero_bias = (
        not np.any(np.asarray(bq)) and not np.any(np.asarray(bk))
        and not np.any(np.asarray(bv))
    )
    # fast path assumes an inverted-dropout mask: two-valued {0, s}
    dmf = np.asarray(drop_mask, np.float32)
    nz = dmf[dmf != 0]
    two_valued = nz.size == 0 or bool(np.all(nz == nz.flat[0]))
    if not (zero_bias and two_valued):
        return _numpy_reference(
            np.asarray(q, np.float32), np.asarray(k, np.float32),
            np.asarray(v, np.float32), np.asarray(Wq, np.float32),
            np.asarray(bq, np.float32), np.asarray(Wk, np.float32),
            np.asarray(bk, np.float32), np.asarray(Wv, np.float32),
            np.asarray(bv, np.float32), np.asarray(drop_mask, np.float32),
        )
    nc = get_nc(fast=True)
    in_maps = make_in_maps_fast(q, k, v, Wq, Wk, Wv, drop_mask)
    res = run_bass_kernel_spmd(nc, in_maps, core_ids=list(range(B)))
    return np.stack(
        [res.results[i]["out"] for i in range(B)], axis=0
    ).astype(np.float32)


# revision 49
# speedup vs baseline: 1.0225x; 1.0225x over previous
"""Trainium2 Bass kernel for single-head attention with pre-softmax score dropout.

Reference computation (per batch element b):
    qp = q @ Wq.T; kp = k @ Wk.T; vp = v @ Wv.T      (biases are zero)
    S  = (qp @ kp.T) / sqrt(D) * drop_mask
    out = softmax(S, axis=-1) @ vp

Sharding: data-parallel over batch B=8 across the 8 NeuronCores (one batch
element per core); weights replicated. No collectives.

Host-side prep (layout only — no activation FLOPs): inputs are shipped
pre-transposed and pre-cast to bf16 (qT/kT/vT), and the two score
projections are constant-folded into one matrix Nw = Wq^T @ Wk (weight-weight
algebra, f32 on host), plus WvT = Wv^T.  This removes every TensorE transpose
and halves HBM traffic vs f32.

Device pipeline per core — TensorE runs ONLY productive N~512 matmuls:
  - qmT[b,t] = sum_a Nw[a,b] qT[a,t]          (64 matmuls)
  - S^T tiles [tk=128, tq=512]: lhsT=kT slice, rhs=qmT chunk (256 matmuls).
    Scores are computed TRANSPOSED so that exp() directly yields P^T, which is
    exactly the stationary operand the PV matmul needs — no P transpose.
  - DVE multiplies S^T by maskT tile; ScalarE computes exp(x/sqrt(D)) -> bf16.
  - vp[t,e] with a ones-column appended ([128, 513] bf16).  PV for each tq
    block runs two accumulation chains: chain A (cols E1:D + ones) first,
    then chain B (cols 0:E1).  The ones-column row-sum drops out of chain A,
    so the reciprocal + chain-A normalize overlap chain B's matmuls and only
    one normalize + a 128KB bf16 store trail the final matmul.

Softmax max-subtraction is skipped deliberately: scores are ~N(0,1) scaled by
at most 1/(1-p)=1.43, so |s| stays far inside f32 exp range.

DMA plan.  Key hardware mechanics (measured): in-flight ring transfers on a
queue share its bandwidth concurrently (~125-150GB/s per HWDGE queue, ~358GB/s
core budget); the completion-semaphore ring (8 sems) is shared across both
HWDGE queues in emission order; a DMA-issue instruction that waits for a ring
slot or a dependency blocks the whole in-order engine queue behind it; DMA
rate needs 2-4KB/partition contiguous descriptors (1KB is ~4x slower).  Hence:
  sync   : qT0, kT pieces g0/g1/g4..g7, wvT, vT half 1, qT2, qT3, [out blocks]
  scalar : Nw, qT1, kT g2/g3, vT half 0 and nothing later — Scalar also runs
           the qmT copies / exp / normalize, so its queue must never carry a
           late-blocking DMA issue.
  gpsimd : mask quarter-chunks [P,4,TCH], fp8 {0,1} in DRAM cast to bf16 in
           flight (mask scale is folded into Nw on the host); mask pool of 4
           bufs makes chunk c+2's loads chain on chunk c's consumption.
kT is re-blocked in DRAM as tkb-PAIR pieces (each its own contiguous
2KB/partition region) split across both queues, so score chunk 0 is never
DMA-stalled.  The phase order qm0,sc0,qm1,sc1,vp,pv0,qm2,sc2,pv1,... pushes
the vT/wvT deadline to ~46us (pt pool holds two chunks of P^T tiles).
A 46-matmul warm-up burns the initial Nw/qT0 DMA wait and holds the PE
clock-ramp (HAM governor) at speed before productive work starts.
"""

import numpy as np
import ml_dtypes

import concourse.bass as bass
import concourse.bacc as bacc
import concourse.mybir as mybir
import concourse.tile as tile
from concourse.bass_utils import run_bass_kernel_spmd

B, T, D, P = 8, 2048, 512, 128
DB = D // P     # 4 blocks of the contraction/projection dims
TB = T // P     # 16 tk row blocks
NCH = 4         # tq chunks
TCH = T // NCH  # 512
QT = 4          # mask quarter: 4 tk-blocks per quarter tile
E1 = 272        # PV split: chain B = [0:E1]; chain A = [E1:D]+ones
F32 = mybir.dt.float32
BF16 = mybir.dt.bfloat16
AF = mybir.ActivationFunctionType
INV_SQRT_D = 1.0 / float(np.sqrt(D))
BF = ml_dtypes.bfloat16

_CACHED = {}


def _build():
    nc = bacc.Bacc("TRN2", target_bir_lowering=False, debug=False, num_devices=B)

    FP8 = mybir.dt.float8e4
    qT_ext = nc.declare_dram_parameter("qTb", [NCH, P, DB, TCH], BF16,
                                       isOutput=False)
    # kT in tkb-pair pieces: [g, p, bb, j2] = k[g*(TCH//2)+j2, bb*P+p]
    kT_ext = nc.declare_dram_parameter("kTb", [2 * NCH, P, DB, TCH // 2], BF16,
                                       isOutput=False)
    vT_ext = nc.declare_dram_parameter("vT", [D, T], BF16, isOutput=False)
    n_ext = nc.declare_dram_parameter("Nwb", [P, DB, DB, P], BF16,
                                      isOutput=False)
    wvT_ext = nc.declare_dram_parameter("WvTb", [P, DB, D], BF16,
                                        isOutput=False)
    # mask, quarter-blocked: [c, p, quarter, j, tq] = maskT[(q*4+j)*P+p, c*TCH+tq]
    mT_ext = nc.declare_dram_parameter("maskT", [NCH, P, NCH, QT, TCH], FP8,
                                       isOutput=False)
    out_ext = nc.declare_dram_parameter("out", [T, D], BF16, isOutput=True)

    with tile.TileContext(nc) as tc:
        with (
            tc.tile_pool(name="wsb", bufs=1) as wsb_pool,
            tc.tile_pool(name="xsb", bufs=1) as xsb_pool,
            tc.tile_pool(name="mask", bufs=4) as mask_pool,
            tc.tile_pool(name="pm", bufs=4) as pm_pool,
            tc.tile_pool(name="pt", bufs=34) as pt_pool,
            tc.tile_pool(name="ob", bufs=3) as ob_pool,
            tc.tile_pool(name="small", bufs=4) as small_pool,
            tc.tile_pool(name="psw", bufs=2, space="PSUM") as psw_pool,
            # pv pools need only 1 buf each: opA's consumers (recip+mulA) run
            # during the opB chain and vice versa, so the next tqb's chain
            # never waits.  The freed banks give the score phase 4 sp bufs,
            # decoupling the PE from DVE/mask-DMA latency.
            tc.tile_pool(name="pss", bufs=4, space="PSUM") as pss_pool,
            tc.tile_pool(name="pso1", bufs=1, space="PSUM") as pso1_pool,
            tc.tile_pool(name="pso2", bufs=1, space="PSUM") as pso2_pool,
        ):
            # ---- DMA in (see module docstring for the pacing plan).
            n_sb = wsb_pool.tile([P, DB, DB, P], BF16, tag="n")
            qT_sb = xsb_pool.tile([P, NCH, DB, TCH], BF16, tag="qT")
            # kT pair-major so each pair DMA is 2KB/partition contiguous on
            # BOTH sides (a strided dst would degrade descriptors to 512B).
            kT_sb = xsb_pool.tile([P, 2 * NCH, DB, TCH // 2], BF16, tag="kT")
            TH2 = TCH // 2

            def load_ktp(g, eng):
                eng.dma_start(kT_sb[:, g], kT_ext[g])

            # In-flight ring transfers on a queue share its bandwidth and
            # the Tile scheduler freely hoists DMA issues, so exact pacing
            # is not controllable — instead keep qT0/Nw first on their
            # queues, spread the kT pieces across both queues in consumption
            # order, and put vT0 on scalar so the vp phase is never gated by
            # the long sync-queue backlog.
            nc.sync.dma_start(qT_sb[:, 0], qT_ext[0])
            nc.scalar.dma_start(n_sb[:], n_ext[:])
            load_ktp(0, nc.sync)
            nc.scalar.dma_start(qT_sb[:, 1], qT_ext[1])
            load_ktp(1, nc.sync)
            load_ktp(2, nc.scalar)
            load_ktp(3, nc.scalar)
            load_ktp(4, nc.sync)
            load_ktp(5, nc.sync)
            load_ktp(6, nc.sync)
            load_ktp(7, nc.sync)
            vT_sb = xsb_pool.tile([P, DB, T], BF16, tag="vT")
            TH = T // 2
            nc.scalar.dma_start(
                vT_sb[:, :, 0:TH],
                vT_ext[0:D, 0:TH].rearrange("(a p) t -> p a t", p=P),
            )
            wvT_sb = wsb_pool.tile([P, DB, D], BF16, tag="wvT")
            nc.sync.dma_start(wvT_sb[:], wvT_ext[:])
            nc.sync.dma_start(
                vT_sb[:, :, TH:T],
                vT_ext[0:D, TH:T].rearrange("(a p) t -> p a t", p=P),
            )
            nc.sync.dma_start(qT_sb[:, 2], qT_ext[2])
            nc.sync.dma_start(qT_sb[:, 3], qT_ext[3])

            masks = {}

            def load_masks(c):
                for q in range(NCH):
                    mk = mask_pool.tile([P, QT, TCH], BF16, tag="mask")
                    nc.gpsimd.dma_start(mk[:], mT_ext[c, :, q])
                    masks[(c, q)] = mk

            load_masks(0)
            load_masks(1)

            qmT_sb = xsb_pool.tile([P, DB, T], BF16, tag="qmT")
            vp_sb = xsb_pool.tile([P, TB, D + 1], BF16, tag="vp")
            nc.vector.memset(vp_sb[:, :, D:D + 1], 1.0)

            # ---- PE warm-up: the HAM clock gate holds the PE at reduced
            # clock until it has seen ~3us of sustained activity, and TensorE
            # is idle waiting on the Nw/qT0 DMA until ~11us anyway — burn the
            # wait on dummy matmuls (memset data, no DRAM dep).
            warm = wsb_pool.tile([P, P], BF16, tag="warm")
            nc.vector.memset(warm[:], 0.5)
            wps = psw_pool.tile([P, P], F32, tag="work")
            for i in range(46):
                nc.tensor.matmul(
                    wps[:], warm[:], warm[:], start=(i == 0), stop=(i == 45)
                )

            def qm_chunk(c):
                for bb in range(DB):
                    work = psw_pool.tile([P, TCH], F32, tag="work")
                    for ab in range(DB):
                        nc.tensor.matmul(
                            work[:],
                            n_sb[:, bb, ab, :],
                            qT_sb[:, c, ab, :],
                            start=(ab == 0),
                            stop=(ab == DB - 1),
                        )
                    # alternate engines so consecutive copies overlap and the
                    # qm->score joint isn't serialized on one engine.
                    # Scalar-first parity: Vector-first measured 3us WORSE —
                    # the scheduler interleaves the next qm into the score
                    # chunk's tail, where a Vector copy queues behind the DVE
                    # muls that feed the score PSUM ring.
                    dst = qmT_sb[:, bb, c * TCH:(c + 1) * TCH]
                    if bb % 2 == 0:
                        nc.scalar.copy(dst, work[:])
                    else:
                        nc.vector.tensor_copy(dst, work[:])

            def score_chunk(c):
                """Returns the 16 P^T tiles [tk=128, tq=TCH] for this chunk."""
                if c + 2 < NCH:
                    load_masks(c + 2)
                pts = []
                for tkb in range(TB):
                    sp = pss_pool.tile([P, TCH], F32, tag="sp")
                    for bb in range(DB):
                        nc.tensor.matmul(
                            sp[:],
                            kT_sb[:, tkb // 2, bb,
                                  (tkb % 2) * P:(tkb % 2 + 1) * P],
                            qmT_sb[:, bb, c * TCH:(c + 1) * TCH],
                            start=(bb == 0),
                            stop=(bb == DB - 1),
                        )
                    pmt = pm_pool.tile([P, TCH], F32, tag="pm")
                    nc.vector.tensor_mul(
                        pmt[:], sp[:], masks[(c, tkb // QT)][:, tkb % QT, :]
                    )
                    pt = pt_pool.tile([P, TCH], BF16, tag="pt")
                    nc.scalar.activation(pt[:], pmt[:], AF.Exp, scale=INV_SQRT_D)
                    pts.append(pt)
                return pts

            def vp_phase():
                for tb in range(TB):
                    work = psw_pool.tile([P, D], F32, tag="work")
                    for db in range(DB):
                        nc.tensor.matmul(
                            work[:],
                            vT_sb[:, db, tb * P:(tb + 1) * P],
                            wvT_sb[:, db, :],
                            start=(db == 0),
                            stop=(db == DB - 1),
                        )
                    nc.vector.tensor_copy(vp_sb[:, tb, 0:D], work[:])

            def pv_chunk(c, pts):
                for tqb in range(NCH):
                    m = c * NCH + tqb
                    # chain A first: cols [E1:D] + the ones column, so the
                    # softmax row-sum (last col) is ready while chain B runs.
                    opA = pso2_pool.tile([P, D - E1 + 1], F32, tag="opA")
                    for tkb in range(TB):
                        nc.tensor.matmul(
                            opA[:], pts[tkb][:, tqb * P:(tqb + 1) * P],
                            vp_sb[:, tkb, E1:D + 1],
                            start=(tkb == 0), stop=(tkb == TB - 1),
                        )
                    rinv = small_pool.tile([P, 1], F32, tag="rinv")
                    nc.vector.reciprocal(rinv[:], opA[:, D - E1:D - E1 + 1])
                    ob = ob_pool.tile([P, D], BF16, tag="ob")
                    nc.scalar.mul(ob[:, E1:D], opA[:, 0:D - E1], rinv[:, 0:1])
                    opB = pso1_pool.tile([P, E1], F32, tag="opB")
                    for tkb in range(TB):
                        nc.tensor.matmul(
                            opB[:], pts[tkb][:, tqb * P:(tqb + 1) * P],
                            vp_sb[:, tkb, 0:E1],
                            start=(tkb == 0), stop=(tkb == TB - 1),
                        )
                    # NOTE: splitting this final normalize across Scalar +
                    # Vector (tensor_scalar_mul with an AP scalar) and the
                    # store across both queues by partition halves looked
                    # like a ~1us tail win but reproducibly slowed EVERY
                    # matmul in the kernel by ~20% (sc 216->259ns) — some
                    # SBUF-port / clock-governor interaction.  Keep the
                    # simple single-engine normalize + one contiguous store.
                    nc.scalar.mul(ob[:, 0:E1], opB[:], rinv[:, 0:1])
                    nc.sync.dma_start(out_ext[m * P:(m + 1) * P, :], ob[:])

            # ---- schedule: keep TensorE dense; sc1 before vp relaxes the
            # vT/wvT DMA deadline; pv(c) recycles sc(c)'s pt tiles just in
            # time for sc(c+2)'s allocations (pt pool holds 2 chunks + slack).
            qm_chunk(0)
            pts0 = score_chunk(0)
            qm_chunk(1)
            pts1 = score_chunk(1)
            vp_phase()
            pv_chunk(0, pts0)
            qm_chunk(2)
            pts2 = score_chunk(2)
            pv_chunk(1, pts1)
            qm_chunk(3)
            pts3 = score_chunk(3)
            pv_chunk(2, pts2)
            pv_chunk(3, pts3)

    nc.compile()
    return nc


def get_nc(fast=True):
    key = "fast"
    if key not in _CACHED:
        _CACHED[key] = _build()
    return _CACHED[key]


def make_in_maps_fast(q, k, v, Wq, Wk, Wv, drop_mask):
    q = np.asarray(q, np.float32)
    k = np.asarray(k, np.float32)
    v = np.asarray(v, np.float32)

    def _block(w):
        # [D, D] -> [P, DB, D] with [p, a, :] = w[a*P + p, :]
        return np.ascontiguousarray(
            w.reshape(DB, P, D).transpose(1, 0, 2).astype(BF)
        )

    def _chunk_block(xT):
        # [D, T] -> [c, p, ab, j] = xT[ab*P + p, c*TCH + j]
        return np.ascontiguousarray(
            xT.reshape(DB, P, NCH, TCH).transpose(2, 1, 0, 3)
        )

    def _pair_block(xT):
        # [D, T] -> [g, p, bb, j2] = xT[bb*P + p, g*(TCH//2) + j2]
        return np.ascontiguousarray(
            xT.reshape(DB, P, 2 * NCH, TCH // 2).transpose(2, 1, 0, 3)
        )

    dm = np.asarray(drop_mask, np.float32)
    # drop_mask is {0, 1/(1-p)}: fold its scale into Nw (weight algebra)
    # and ship the mask itself as exact {0,1} fp8.
    mask_scale = float(dm.max()) if dm.size else 1.0
    if mask_scale == 0.0:
        mask_scale = 1.0
    # [a, b] -> [p, bb, ab, j] = Nw[ab*P + p, bb*P + j]  (4KB/partition DMA)
    Nw_f = (
        np.asarray(Wq, np.float32).T @ np.asarray(Wk, np.float32)
    ) * np.float32(mask_scale)
    Nw = np.ascontiguousarray(
        Nw_f.reshape(DB, P, DB, P).transpose(1, 2, 0, 3).astype(BF)
    )
    WvT = _block(np.asarray(Wv, np.float32).T.copy())
    F8 = ml_dtypes.float8_e4m3
    maps = []
    for i in range(B):
        # maskT[tk, tq] -> [c, p, quarter, j, tq], tk = (quarter*QT+j)*P + p
        mt = (dm[i].T != 0).astype(F8)
        mtb = np.ascontiguousarray(
            mt.reshape(NCH, QT, P, NCH, TCH).transpose(3, 2, 0, 1, 4)
        )
        maps.append({
            "qTb": _chunk_block(q[i].T.astype(BF)),
            "kTb": _pair_block(k[i].T.astype(BF)),
            "vT": np.ascontiguousarray(v[i].T.astype(BF)),
            "Nwb": Nw,
            "WvTb": WvT,
            "maskT": mtb,
        })
    return maps


def _numpy_reference(q, k, v, Wq, bq, Wk, bk, Wv, bv, drop_mask):
    """Correctness fallback for nonzero biases (never hit by setup_inputs)."""
    qp = np.einsum("btd,ed->bte", q, Wq) + bq
    kp = np.einsum("btd,ed->bte", k, Wk) + bk
    vp = np.einsum("btd,ed->bte", v, Wv) + bv
    score = np.einsum("bqd,bkd->bqk", qp, kp) / np.sqrt(np.float32(D))
    score = score * drop_mask
    score -= score.max(axis=-1, keepdims=True)
    e = np.exp(score)
    attn = e / e.sum(axis=-1, keepdims=True)
    return np.einsum("bqk,bkd->bqd", attn, vp).astype(np.float32)


def kernel(q, k, v, Wq, bq, Wk, bk, Wv, bv, drop_mask):
    zero_bias = (
        not np.any(np.asarray(bq)) and not np.any(np.asarray(bk))
        and not np.any(np.asarray(bv))
    )
    # fast path assumes an inverted-dropout mask: two-valued {0, s}
    dmf = np.asarray(drop_mask, np.float32)
    nz = dmf[dmf != 0]
    two_valued = nz.size == 0 or bool(np.all(nz == nz.flat[0]))
    if not (zero_bias and two_valued):
        return _numpy_reference(
            np.asarray(q, np.float32), np.asarray(k, np.float32),
            np.asarray(v, np.float32), np.asarray(Wq, np.float32),
            np.asarray(bq, np.float32), np.asarray(Wk, np.float32),
            np.asarray(bk, np.float32), np.asarray(Wv, np.float32),
            np.asarray(bv, np.float32), np.asarray(drop_mask, np.float32),
        )
    nc = get_nc(fast=True)
    in_maps = make_in_maps_fast(q, k, v, Wq, Wk, Wv, drop_mask)
    res = run_bass_kernel_spmd(nc, in_maps, core_ids=list(range(B)))
    return np.stack(
        [res.results[i]["out"] for i in range(B)], axis=0
    ).astype(np.float32)
